# revision 41
# baseline (speedup 1.0000x reference)
"""TRN2 Bass kernel for block-sparse attention (nn_BlockSparseAttention).

kernel(**inputs) takes the FULL unsharded inputs (x [4,4096,1024], Wq/Wk/Wv/Wo
[1024,1024], bq/bk/bv/bo [1024]) and returns the full output [4,4096,1024].

Sharding: 8 cores = 4 batches x 2 head-halves (8 heads each). Each core
computes QKV projections, block-sparse attention, and a partial
out-projection [4096,1024]; the host sums the two half-partials plus bo.

v3 design (fused single pipeline):
  - projections and attention are interleaved over 8 token chunks of 512:
    the 512-col projection matmuls keep the PE HAM clock-gate at 8/8
    (2.4 GHz) and hide the Act exp / DVE division latency of the
    attention stages, which previously ran in a separate phase at K=4/8
  - S^T matmuls (64-row contraction) for the two head-halves are emitted
    interleaved so they run CONCURRENTLY in disjoint PE row-groups
    (tile_position auto-derived from base_partition) -- ~2x on S
  - no mask matmuls: every S matmul is its own accumulation group and the
    invalid 64x64 window corners are zeroed on the idle GpSimd engine
    after the exp
  - block-0 global attention is split into small (j, 4-piece) tasks spread
    across the whole pipeline; a few out-projections are deferred to keep
    the PE dense through the serial tail (g0 / edge group)
  - HAM pre-warm: dummy matmuls on the identity tile during the initial
    weight/x DMA wait
"""
import os

import numpy as np

import concourse.bass as bass
import concourse.tile as tile
from concourse import mybir

F32 = mybir.dt.float32
F16 = mybir.dt.float16
BF16 = mybir.dt.bfloat16
AF = mybir.ActivationFunctionType
SCALE = 1.0 / 8.0  # 1/sqrt(Dh=64)

N_CORES = 8
LAST_EXEC_NS = None


def _split_sync_waits(nc, cap=1):
    """This walrus build rejects >cap sync waits on one instruction; move
    excess waits onto same-engine no-ops placed just before (waits only
    become stricter in order, so this is semantics-preserving)."""
    for fn in nc.m.functions:
        for bb in fn.blocks:
            out = []
            for inst in bb.instructions:
                si = inst.sync_info
                waits = list(si.on_wait) if si and si.on_wait else []
                if len(waits) > cap:
                    extra, keep = waits[:-cap], waits[-cap:]
                    for i in range(0, len(extra), cap):
                        nop = mybir.InstNoOp(
                            name=nc.get_next_instruction_name(),
                            engine=inst.engine,
                            ins=[],
                            outs=[],
                            sync_info=mybir.SyncInfo(
                                on_wait=extra[i : i + cap], on_update=[]
                            ),
                        )
                        nc.register_instruction(nop)
                        out.append(nop)
                    si.on_wait = keep
                out.append(inst)
            bb.instructions[:] = out


def build_kernel(NT=4096, DM=1024, HL=8, DMO=1024):
    """One-core program; SPMD across 8 cores with different input slices."""
    DO = HL * 64          # local head dims (512)
    DOV = HL * 65         # v with interleaved ones columns (520)
    KC = DM // 128        # d_model chunks (8)
    NJ = DO // 128        # head pairs (4)
    STOK = 512
    NS = NT // STOK       # 8
    NG = NT // 128        # 32 token chunks / query groups
    NCH = NG + 1          # 33 shifted v chunks

    nc = bass.Bass()
    # cache-buster: some compile caches in this stack key on the HLO
    # interface only (not the embedded BIR), so a stale executable from a
    # previous kernel version can be wrongly reused. A source-hash-sized
    # dummy input makes every kernel edit change the HLO signature.
    import hashlib
    _nonce = 1 + int(hashlib.sha256(open(__file__, "rb").read()).hexdigest(), 16) % 509
    nonce_d = nc.dram_tensor("nonce", [1, _nonce], F32, kind="ExternalInput")
    xt_d = nc.dram_tensor("xt", [DM, NT], F16, kind="ExternalInput")
    wq_d = nc.dram_tensor("wq", [DM, DO], F16, kind="ExternalInput")
    wk_d = nc.dram_tensor("wk", [DM, DO], F16, kind="ExternalInput")
    wv_d = nc.dram_tensor("wvp", [DM, DOV], F16, kind="ExternalInput")
    wo_d = nc.dram_tensor("wo", [DO, DMO], BF16, kind="ExternalInput")
    bq_d = nc.dram_tensor("bq", [128, NJ], F32, kind="ExternalInput")
    bk_d = nc.dram_tensor("bk", [128, NJ], F32, kind="ExternalInput")
    idn_d = nc.dram_tensor("idn", [128, 128], BF16, kind="ExternalInput")
    y_d = nc.dram_tensor("y", [NT, DMO], F32, kind="ExternalOutput")

    with tile.TileContext(nc) as tc, nc.allow_low_precision(
        reason="attention operands intentionally bf16/fp16; matmul accum f32"
    ):
        from contextlib import ExitStack

        with ExitStack() as ctx:
            pers = ctx.enter_context(tc.tile_pool(name="pers", bufs=1))
            xp = ctx.enter_context(tc.tile_pool(name="xp", bufs=2))
            etp = ctx.enter_context(tc.tile_pool(name="etp", bufs=24))
            valp = ctx.enter_context(tc.tile_pool(name="valp", bufs=4))
            astp = ctx.enter_context(tc.tile_pool(name="astp", bufs=10))
            atp = ctx.enter_context(tc.tile_pool(name="atp", bufs=14))
            dip = ctx.enter_context(tc.tile_pool(name="dip", bufs=8))
            ysp = ctx.enter_context(tc.tile_pool(name="ysp", bufs=4))
            psp = ctx.enter_context(
                tc.tile_pool(name="psp", bufs=2, space="PSUM")
            )

            kts = [pers.tile([128, NT], BF16, tag=f"kt{j}", name=f"kt{j}")
                   for j in range(NJ)]
            qts = [pers.tile([128, NT], BF16, tag=f"qt{j}", name=f"qt{j}")
                   for j in range(NJ)]
            # shifted v: chunk m = tokens [128m-64, 128m+64); per-j cols 130
            vs = pers.tile([128, NCH * DOV], BF16, tag="vs")
            idn = pers.tile([128, 128], BF16, tag="idn")
            # block-0 PV accumulators (SBUF, accumulated task-wise)
            oq0 = [pers.tile([64, 130], F32, tag=f"oq{j}", name=f"oq{j}")
                   for j in range(NJ)]
            wqs = pers.tile([128, KC * DO], F16, tag="wqs")
            wks = pers.tile([128, KC * DO], F16, tag="wks")
            wvs = pers.tile([128, KC * DOV], F16, tag="wvs")
            bqs = pers.tile([128, NJ], F32, tag="bqs")
            bks = pers.tile([128, NJ], F32, tag="bks")
            wos = [pers.tile([128, DMO], BF16, tag=f"wo{j}", name=f"wo{j}")
                   for j in range(NJ)]

            # ---------------- prologue DMAs ----------------
            nc.sync.dma_start(idn[:], idn_d[:])
            xts_tiles = {}
            xts_tiles[0] = xp.tile([128, KC * STOK], F16, tag="xts",
                                   name="xts0")
            for c in range(KC):
                nc.sync.dma_start(
                    xts_tiles[0][:, c * STOK : (c + 1) * STOK],
                    xt_d[c * 128 : (c + 1) * 128, 0:STOK],
                )
            for c in range(KC):
                r = slice(c * 128, (c + 1) * 128)
                nc.sync.dma_start(wqs[:, c * DO : (c + 1) * DO], wq_d[r, :])
            nc.sync.dma_start(bqs[:], bq_d[:])
            for c in range(KC):
                r = slice(c * 128, (c + 1) * 128)
                nc.sync.dma_start(wks[:, c * DO : (c + 1) * DO], wk_d[r, :])
            nc.sync.dma_start(bks[:], bk_d[:])
            for c in range(KC):
                r = slice(c * 128, (c + 1) * 128)
                nc.sync.dma_start(wvs[:, c * DOV : (c + 1) * DOV], wv_d[r, :])
            for j in range(NJ):
                nc.sync.dma_start(wos[j][:], wo_d[j * 128 : (j + 1) * 128, :])
            # zero the phantom halves of the shifted-v layout so padded
            # full-row PV chains multiply by 0 instead of garbage
            nc.vector.memset(vs[0:64, 0:DOV], 0.0)
            nc.vector.memset(vs[64:128, NG * DOV : (NG + 1) * DOV], 0.0)

            # HAM pre-warm: ~3.4us of dummy matmuls during the DMA wait so
            # the first real chains run at K=8/8. Uses the first psY slot;
            # nothing reads it.
            NPW = int(os.environ.get("BSV3_PREWARM", "40"))
            if NPW:
                pw = psp.tile([128, 512], F32, tag="psYT", name="prewarm",
                              bufs=2)
                for i in range(NPW):
                    nc.tensor.matmul(pw[:, 0:128], idn[:], idn[:],
                                     start=(i == 0), stop=(i == NPW - 1))

            # ---------------- helpers ----------------
            def vcol(m, j, hh):
                return m * DOV + j * 130 + hh * 65

            ets_store = {}  # pair g1 -> {(j, hh): (tile, qoff)}
            as_tiles = {}   # g -> [ast per j]
            at_tiles = {}   # g -> at tile

            def emit_S2j(g1, j):
                """S^T for group pair (g1, g1+1), head pair j, both halves
                interleaved so the two 64-row chains run concurrently in
                disjoint PE row-groups. Tile layout per (j, hh):
                [ca x q1 | cm x q1 | cm x q2 | cb x q2], 128 cols each.
                Every matmul is its own accumulation group (no column
                overlap); the 4 invalid 64x64 corners are zeroed by GpSimd
                after the exp."""
                g2 = g1 + 1
                ka = 128 * g1 - 64
                km = 128 * g1 + 64
                kb = 128 * g2 + 64
                ps = {}
                for hh in (0, 1):
                    ps[hh] = psp.tile([128, 512], F32, tag="psS",
                                      name=f"psS_{g1}_{j}_{hh}")
                if os.environ.get("BSV3_SEQS2"):
                    order = [(seg, hh) for hh in (0, 1) for seg in range(3)]
                else:
                    order = [(seg, hh) for seg in range(3) for hh in (0, 1)]
                segs = ((ka, 0, 128), (km, 0, 256), (kb, 128, 128))
                for seg, hh in order:
                    ko, qo, qn = segs[seg]
                    co = (0, 128, 384)[seg]
                    hr = slice(hh * 64, hh * 64 + 64)
                    nc.tensor.matmul(
                        ps[hh][:, co : co + qn],
                        kts[j][hr, ko : ko + 128],
                        qts[j][hr, 128 * g1 + qo : 128 * g1 + qo + qn],
                        start=True, stop=True,
                    )
                st = ets_store.setdefault(g1, {})
                MS = os.environ.get("BSV3_MEMSET", "gpsimd")
                for hh in (0, 1):
                    et = etp.tile([128, 512], BF16, tag="et",
                                  name=f"et_{g1}_{j}_{hh}")
                    nc.scalar.activation(et[:], ps[hh][:], AF.Exp,
                                         scale=SCALE)
                    eng = {"gpsimd": nc.gpsimd, "vector": nc.vector,
                           "none": None}[MS]
                    if eng is not None:
                        eng.memset(et[0:64, 64:128], 0.0)
                        eng.memset(et[64:128, 128:192], 0.0)
                        eng.memset(et[0:64, 320:384], 0.0)
                        eng.memset(et[64:128, 384:448], 0.0)
                    st[(j, hh)] = (et, 0)
                    st[(j, hh, "g2")] = (et, 256)

            def emit_S31j(j):
                """Edge group 31 (no right neighbor), head pair j. Layout
                per hh half-tile: [ca x q | cb x q], cb only 64 keys."""
                g = NG - 1
                ka = 128 * g - 64
                kb = 128 * g + 64
                ps = {}
                for hh in (0, 1):
                    ps[hh] = psp.tile([128, 512], F32, tag="psS",
                                      name=f"psS31_{j}_{hh}")
                # adjacent matmuls in disjoint row-groups run concurrently
                # and must drain to different PSUM banks
                for hh in (0, 1):
                    hr = slice(hh * 64, hh * 64 + 64)
                    nc.tensor.matmul(
                        ps[hh][:, 0:128],
                        kts[j][hr, ka : ka + 128],
                        qts[j][hr, 128 * g : 128 * g + 128],
                        start=True, stop=True,
                    )
                for hh in (0, 1):
                    hr = slice(hh * 64, hh * 64 + 64)
                    nc.tensor.matmul(
                        ps[hh][0:64, 128:256],
                        kts[j][hr, kb : kb + 64],
                        qts[j][hr, 128 * g : 128 * g + 128],
                        start=True, stop=True,
                    )
                st = ets_store.setdefault("e31", {})
                et = etp.tile([128, 512], BF16, tag="et", name=f"et31_{j}")
                for hh in (0, 1):
                    o = hh * 256
                    nc.scalar.activation(et[:, o : o + 128],
                                         ps[hh][:, 0:128],
                                         AF.Exp, scale=SCALE)
                    nc.scalar.activation(et[0:64, o + 128 : o + 256],
                                         ps[hh][0:64, 128:256],
                                         AF.Exp, scale=SCALE)
                    nc.gpsimd.memset(et[0:64, o + 64 : o + 128], 0.0)
                    nc.gpsimd.memset(et[64:128, o + 128 : o + 256], 0.0)
                    st[(j, hh)] = (et, hh * 256)
                return st

            def emit_PV(g, ets):
                """merged PV, batched reciprocal, division on DVE."""
                as_tiles[g] = []
                for j in range(NJ):
                    jj = j % 2
                    if jj == 0:
                        pv = psp.tile([128, 512], F32, tag="psPV",
                                      name=f"psPV_{g}_{j}")
                    c0 = jj * 130
                    for hh in (0, 1):
                        et, off = ets[(j, hh)]
                        co = c0 + hh * 65
                        nc.tensor.matmul(
                            pv[:, co : co + 65],
                            et[:, off : off + 128],
                            vs[:, vcol(g, j, hh) : vcol(g, j, hh) + 65],
                            start=True, stop=False,
                        )
                        nc.tensor.matmul(
                            pv[:, co : co + 65],
                            et[:, off + 128 : off + 256],
                            vs[:, vcol(g + 1, j, hh) : vcol(g + 1, j, hh) + 65],
                            start=False, stop=True,
                        )
                    if jj == 1:
                        dinv = dip.tile([128, 4], F32, tag="dinv",
                                        name=f"dinv_{g}_{j}")
                        nc.vector.reciprocal(
                            dinv[:],
                            pv[:, 0:260].rearrange(
                                "p (h c) -> p h c", c=65
                            )[:, :, 64:65],
                        )
                        for j2 in (j - 1, j):
                            ast = astp.tile([128, 128], BF16, tag="ast",
                                            name=f"ast_{g}_{j2}")
                            cb = (j2 % 2) * 130
                            for hh in (0, 1):
                                nc.vector.tensor_scalar_mul(
                                    ast[:, hh * 64 : hh * 64 + 64],
                                    pv[:, cb + hh * 65 : cb + hh * 65 + 64],
                                    dinv[:, (j2 % 2) * 2 + hh : (j2 % 2) * 2 + hh + 1],
                                )
                            as_tiles[g].append(ast)

            def emit_T(g):
                """PE-transpose astage -> one A^T tile [128, 4*128]."""
                pt = psp.tile([128, 1024], BF16, tag="psYT",
                              name=f"psT_{g}", bufs=2)
                for j in range(NJ):
                    nc.tensor.transpose(
                        pt[:, j * 128 : (j + 1) * 128], as_tiles[g][j], idn
                    )
                att = atp.tile([128, 512], BF16, tag="at", name=f"at_{g}")
                nc.vector.tensor_copy(att[:], pt[:, 0:512])
                at_tiles[g] = att
                del as_tiles[g]

            def emit_outproj_half(g, n):
                py = psp.tile([128, 512], F32, tag="psYT",
                              name=f"psY_{g}_{n}", bufs=2)
                for j in range(NJ):
                    nc.tensor.matmul(
                        py[:],
                        at_tiles[g][:, j * 128 : (j + 1) * 128],
                        wos[j][:, n * 512 : n * 512 + 512],
                        start=(j == 0), stop=(j == NJ - 1),
                    )
                ysb = ysp.tile([128, 512], F32, tag="ysb", name=f"ysb_{g}_{n}")
                if (g + n) % 2 == 0:
                    nc.scalar.copy(ysb[:], py[:])
                else:
                    nc.vector.tensor_copy(ysb[:], py[:])
                nc.sync.dma_start(
                    y_d[g * 128 : (g + 1) * 128, n * 512 : n * 512 + 512],
                    ysb[:],
                )
                if n == 1:
                    del at_tiles[g]

            # ----- block-0 global attention tasks -----
            def q0_edge0(j):
                """piece 0 (keys 0:64 live on partitions 64:128 of chunk 0);
                initializes oq0[j] via tensor_copy."""
                ps = {}
                for hh in (0, 1):
                    ps[hh] = psp.tile([128, 512], F32, tag="psS",
                                      name=f"q0e0_{j}_{hh}")
                    hr = slice(hh * 64, hh * 64 + 64)
                    nc.tensor.matmul(
                        ps[hh][64:128, 0:64],
                        kts[j][hr, 0:64], qts[j][hr, 0:64],
                        start=True, stop=True,
                    )
                eq = etp.tile([128, 512], BF16, tag="et", name=f"eq0_{j}")
                for hh in (0, 1):
                    nc.scalar.activation(eq[64:128, hh * 64 : hh * 64 + 64],
                                         ps[hh][64:128, 0:64],
                                         AF.Exp, scale=SCALE)
                pv = psp.tile([128, 512], F32, tag="psPV", name=f"pvq0_{j}")
                for hh in (0, 1):
                    nc.tensor.matmul(
                        pv[0:64, hh * 65 : hh * 65 + 65],
                        eq[64:128, hh * 64 : hh * 64 + 64],
                        vs[64:128, vcol(0, j, hh) : vcol(0, j, hh) + 65],
                        start=True, stop=True,
                    )
                nc.vector.tensor_copy(oq0[j][:], pv[0:64, 0:130])

            def q0_edge32(j):
                """piece 32 (keys 4032:4096 on partitions 0:64 of chunk 32)."""
                ps = {}
                for hh in (0, 1):
                    ps[hh] = psp.tile([128, 512], F32, tag="psS",
                                      name=f"q0e32_{j}_{hh}")
                    hr = slice(hh * 64, hh * 64 + 64)
                    nc.tensor.matmul(
                        ps[hh][0:64, 0:64],
                        kts[j][hr, NT - 64 : NT], qts[j][hr, 0:64],
                        start=True, stop=True,
                    )
                eq = etp.tile([128, 512], BF16, tag="et", name=f"eq32_{j}")
                for hh in (0, 1):
                    nc.scalar.activation(eq[0:64, hh * 64 : hh * 64 + 64],
                                         ps[hh][0:64, 0:64],
                                         AF.Exp, scale=SCALE)
                pv = psp.tile([128, 512], F32, tag="psPV", name=f"pvq32_{j}")
                for hh in (0, 1):
                    nc.tensor.matmul(
                        pv[0:64, hh * 65 : hh * 65 + 65],
                        eq[0:64, hh * 64 : hh * 64 + 64],
                        vs[0:64, vcol(NG, j, hh) : vcol(NG, j, hh) + 65],
                        start=True, stop=True,
                    )
                nc.vector.tensor_add(oq0[j][:], oq0[j][:], pv[0:64, 0:130])

            def q0_range(j, r):
                """block-0 vs pieces 4r+1..4r+4 (clipped to 31) for head
                pair j. S matmuls packed across hh row-groups; one exp; PV
                chains per hh accumulated into oq0[j]."""
                pieces = [m for m in range(4 * r + 1, 4 * r + 5) if m <= NG - 1]
                ps = {}
                for hh in (0, 1):
                    ps[hh] = psp.tile([128, 512], F32, tag="psS",
                                      name=f"q0r_{j}_{r}_{hh}")
                for c, m in enumerate(pieces):
                    for hh in (0, 1):
                        hr = slice(hh * 64, hh * 64 + 64)
                        nc.tensor.matmul(
                            ps[hh][:, c * 64 : c * 64 + 64],
                            kts[j][hr, 128 * m - 64 : 128 * m + 64],
                            qts[j][hr, 0:64],
                            start=True, stop=True,
                        )
                eq = etp.tile([128, 512], BF16, tag="et", name=f"eqr_{j}_{r}")
                nw = len(pieces) * 64
                for hh in (0, 1):
                    nc.scalar.activation(
                        eq[:, hh * 256 : hh * 256 + nw],
                        ps[hh][:, 0:nw],
                        AF.Exp, scale=SCALE,
                    )
                pv = psp.tile([128, 512], F32, tag="psPV", name=f"pvr_{j}_{r}")
                for hh in (0, 1):
                    for c, m in enumerate(pieces):
                        nc.tensor.matmul(
                            pv[0:64, hh * 65 : hh * 65 + 65],
                            eq[:, hh * 256 + c * 64 : hh * 256 + c * 64 + 64],
                            vs[:, vcol(m, j, hh) : vcol(m, j, hh) + 65],
                            start=(c == 0), stop=(c == len(pieces) - 1),
                        )
                nc.vector.tensor_add(oq0[j][:], oq0[j][:], pv[0:64, 0:130])

            def emit_g0():
                """group 0: block 0 (rows 0:64, from the accumulated global
                pass) + block 1 (rows 64:128, local window {0,1,2})."""
                as_tiles[0] = []
                ega, egb = {}, {}
                for hh in (0, 1):
                    hr = slice(hh * 64, hh * 64 + 64)
                    psA = psp.tile([128, 512], F32, tag="psS",
                                   name=f"g0a_{hh}")
                    for j in range(NJ):
                        nc.tensor.matmul(
                            psA[64:128, j * 64 : j * 64 + 64],
                            kts[j][hr, 0:64], qts[j][hr, 64:128],
                            start=True, stop=True,
                        )
                        nc.tensor.matmul(
                            psA[:, 256 + j * 64 : 256 + j * 64 + 64],
                            kts[j][hr, 64:192], qts[j][hr, 64:128],
                            start=True, stop=True,
                        )
                    ea = etp.tile([128, 512], BF16, tag="et", name=f"ga_{hh}")
                    nc.scalar.activation(
                        ea[64:128, 0:256], psA[64:128, 0:256],
                        AF.Exp, scale=SCALE,
                    )
                    nc.scalar.activation(ea[:, 256:512], psA[:, 256:512],
                                         AF.Exp, scale=SCALE)
                    nc.gpsimd.memset(ea[0:64, 0:256], 0.0)
                    ega[hh], egb[hh] = ea, ea
                for j in range(NJ):
                    jj = j % 2
                    if jj == 0:
                        pv0 = psp.tile([128, 512], F32, tag="psPV",
                                       name=f"pv0_{j}")
                    c0 = jj * 130
                    for hh in (0, 1):
                        nc.tensor.matmul(
                            pv0[64:128, c0 + hh * 65 : c0 + hh * 65 + 65],
                            ega[hh][:, j * 64 : j * 64 + 64],
                            vs[:, vcol(0, j, hh) : vcol(0, j, hh) + 65],
                            start=True, stop=False,
                        )
                        nc.tensor.matmul(
                            pv0[64:128, c0 + hh * 65 : c0 + hh * 65 + 65],
                            egb[hh][:, 256 + j * 64 : 256 + j * 64 + 64],
                            vs[:, vcol(1, j, hh) : vcol(1, j, hh) + 65],
                            start=False, stop=True,
                        )
                    dinv = dip.tile([128, 4], F32, tag="dinv",
                                    name=f"dinv0_{j}")
                    for hh in (0, 1):
                        nc.vector.reciprocal(
                            dinv[64:128, hh : hh + 1],
                            pv0[64:128, c0 + hh * 65 + 64 : c0 + hh * 65 + 65],
                        )
                        nc.vector.reciprocal(
                            dinv[0:64, hh : hh + 1],
                            oq0[j][:, hh * 65 + 64 : hh * 65 + 65],
                        )
                    ast = astp.tile([128, 128], BF16, tag="ast",
                                    name=f"ast0_{j}")
                    for hh in (0, 1):
                        nc.vector.tensor_scalar_mul(
                            ast[64:128, hh * 64 : hh * 64 + 64],
                            pv0[64:128, c0 + hh * 65 : c0 + hh * 65 + 64],
                            dinv[64:128, hh : hh + 1],
                        )
                        nc.vector.tensor_scalar_mul(
                            ast[0:64, hh * 64 : hh * 64 + 64],
                            oq0[j][:, hh * 65 : hh * 65 + 64],
                            dinv[0:64, hh : hh + 1],
                        )
                    as_tiles[0].append(ast)

            # ---------------- task scheduler ----------------
            # slot L = 16*s + i; tasks emit after the slot's proj chain when
            # ready_L <= L. Insertion order respects intra-group deps.
            tasks = []
            LEVEL = int(os.environ.get("BSV3_LEVEL", "99"))

            def add(ready, cost, fn, lvl=0):
                if lvl > LEVEL:
                    return
                tasks.append([ready, cost, fn, False])

            def pump(L, budget):
                spent = 0.0
                for t in tasks:
                    if t[3] or t[0] > L:
                        continue
                    t[2]()
                    t[3] = True
                    spent += t[1]
                    if spent >= budget:
                        break

            def vslot(T):
                """slot index after which vs chunk T's scatter DMA is
                emitted (+1 slot of lag for the DMA to land). Interleaved
                order: V half (t, oi) sits at slot 2*(2t+oi)+1."""
                s, t = T // 4, T % 4
                return 16 * s + 4 * t + 4

            DEFER = {21, 22, 23, 24}


            def pv1_ets(g1):
                return {k: v for k, v in ets_store[g1].items()
                        if len(k) == 2}

            def pv2_ets(g1):
                return {(j, hh): ets_store[g1][(j, hh, "g2")]
                        for j in range(NJ) for hh in (0, 1)}

            # pair tasks
            for g1 in range(1, NG - 1, 2):
                g2 = g1 + 1
                sready = 16 * ((g2 + 1) // 4) + 15
                for j in range(NJ):
                    add(sready + j // 2, 0.45,
                        (lambda g1=g1, j=j: emit_S2j(g1, j)), lvl=1)
                pvr1 = max(sready + 3, vslot(g1 + 1) + 1)
                pvr2 = max(pvr1 + 1, vslot(g2 + 1) + 1)
                add(pvr1, 0.6,
                    (lambda g1=g1: emit_PV(g1, pv1_ets(g1))), lvl=2)
                add(pvr2, 0.6,
                    (lambda g1=g1, g2=g2: emit_PV(g2, pv2_ets(g1))), lvl=2)
                add(pvr2 + 1, 0.3, (lambda g1=g1: emit_T(g1)), lvl=3)
                add(pvr2 + 1, 0.3, (lambda g2=g2: emit_T(g2)), lvl=3)
                for gg, base in ((g1, pvr2 + 3), (g2, pvr2 + 4)):
                    if gg in DEFER:
                        base = 129 + 2 * (gg - 21)
                    add(base, 0.5,
                        (lambda gg=gg: emit_outproj_half(gg, 0)), lvl=4)
                    add(base + 1, 0.5,
                        (lambda gg=gg: emit_outproj_half(gg, 1)), lvl=4)

            # edge group 31
            S31LVL = 9 if os.environ.get("BSV3_NOS31") else 1
            for j in range(NJ):
                add(16 * 7 + 15 + j // 2, 0.35, (lambda j=j: emit_S31j(j)),
                    lvl=S31LVL)
            add(vslot(NG - 1) + 2, 0.6, (lambda: emit_PV(
                NG - 1, {k: v for k, v in ets_store["e31"].items()})), lvl=2)
            add(vslot(NG - 1) + 3, 0.3, (lambda: emit_T(NG - 1)), lvl=3)
            add(vslot(NG - 1) + 4, 0.5,
                (lambda: emit_outproj_half(NG - 1, 0)), lvl=4)
            add(vslot(NG - 1) + 5, 0.5,
                (lambda: emit_outproj_half(NG - 1, 1)), lvl=4)

            # block-0 tasks
            for j in range(NJ):
                add(16 + j // 2, 0.2, (lambda j=j: q0_edge0(j)), lvl=5)
            for r in range(8):
                for j in range(NJ):
                    rd = max(16 * min(r + 1, NS - 1) + 15,
                             vslot(min(4 * r + 4, NG - 1)) + 1)
                    add(rd + j, 0.45, (lambda j=j, r=r: q0_range(j, r)),
                        lvl=5)
            for j in range(NJ):
                add(vslot(NG - 1) + 2 + j, 0.2, (lambda j=j: q0_edge32(j)),
                    lvl=5)
            add(137, 2.0, emit_g0, lvl=6)
            add(138, 0.3, (lambda: emit_T(0)), lvl=6)
            add(139, 0.5, (lambda: emit_outproj_half(0, 0)), lvl=6)
            add(140, 0.5, (lambda: emit_outproj_half(0, 1)), lvl=6)

            # ---------------- the fused pipeline ----------------
            for s in range(NS):
                ts = slice(s * STOK, (s + 1) * STOK)
                L0 = 16 * s
                if s + 1 < NS:
                    xts_tiles[s + 1] = xp.tile([128, KC * STOK], F16,
                                               tag="xts", name=f"xts{s + 1}")
                    for c in range(KC):
                        nc.sync.dma_start(
                            xts_tiles[s + 1][:, c * STOK : (c + 1) * STOK],
                            xt_d[c * 128 : (c + 1) * 128,
                                 (s + 1) * STOK : (s + 2) * STOK],
                        )
                xts = xts_tiles.pop(s)
                # interleave QK and V chains: every V matmul (stream 108ns
                # ~= its 107ns LDWEIGHTS, zero slack) follows a QK chain
                # whose 213ns-stream matmuls donate LDW-prefetch headroom,
                # so attention-task LDW bursts no longer starve V
                qk_list = [(qi, j) for qi in range(2) for j in range(NJ)]
                val_t = {}
                for i in range(8):
                    qi, j = qk_list[i]
                    wsb, bsb, dsts = ((wqs, bqs, qts), (wks, bks, kts))[qi]
                    ps = psp.tile([128, 512], F32, tag="pp",
                                  name=f"pp_{s}_{qi}_{j}")
                    for c in range(KC):
                        nc.tensor.matmul(
                            ps[:],
                            wsb[:, c * DO + j * 128 : c * DO + (j + 1) * 128],
                            xts[:, c * STOK : (c + 1) * STOK],
                            start=(c == 0), stop=(c == KC - 1),
                        )
                    nc.scalar.activation(
                        dsts[j][:, ts], ps[:], AF.Identity,
                        bias=bsb[:, j : j + 1],
                    )
                    pump(L0 + 2 * i, 1.1)
                    t, oi = i // 2, i % 2
                    T = 4 * s + t
                    if oi == 0:
                        val_t[t] = valp.tile([128, DOV], BF16, tag="val",
                                             name=f"val_{T}")
                    val = val_t[t]
                    o, wd = ((0, 260), (260, 260))[oi]
                    psv = psp.tile([128, 512], F32, tag="pp",
                                   name=f"ppv_{T}_{oi}")
                    for c in range(KC):
                        nc.tensor.matmul(
                            psv[:, 0:wd],
                            xts[:, c * STOK + t * 128 : c * STOK + (t + 1) * 128],
                            wvs[:, c * DOV + o : c * DOV + o + wd],
                            start=(c == 0), stop=(c == KC - 1),
                        )
                    nc.vector.tensor_copy(val[:, o : o + wd], psv[:, 0:wd])
                    pump(L0 + 2 * i + 1, 0.9)
                    if oi == 1:
                        nc.gpsimd.memset(
                            val.rearrange("p (h c) -> p h c", c=65)[:, :, 64:65],
                            1.0,
                        )
                        nc.sync.dma_start(
                            vs[64:128, T * DOV : (T + 1) * DOV], val[0:64, :]
                        )
                        nc.sync.dma_start(
                            vs[0:64, (T + 1) * DOV : (T + 2) * DOV],
                            val[64:128, :]
                        )
            # ---------------- tail ----------------
            L = 16 * NS
            while any(not t[3] for t in tasks):
                pump(L, 2.2)
                L += 1
                assert L < 400, "scheduler deadlock"

    return _finish(nc)


def _finish(nc):
    _split_sync_waits(nc)
    return nc


# ---------------------------------------------------------------- host glue
def shard_inputs(x, Wq, bq, Wk, bk, Wv, bv, Wo, bo):
    """Full inputs -> per-core in_maps. Core c: batch c//2, head-half c%2."""
    import ml_dtypes

    DM = Wq.shape[0]
    DO = Wq.shape[1] // 2
    HL = DO // 64
    DOV = HL * 65
    NJ = DO // 128
    in_maps = []
    cache = {}
    idn = np.eye(128, dtype=ml_dtypes.bfloat16)
    for core in range(N_CORES):
        b, g = core // 2, core % 2
        if g not in cache:
            sl = slice(g * DO, (g + 1) * DO)
            wvp = np.zeros((DM, DOV), np.float32)
            for h in range(HL):
                wvp[:, h * 65 : h * 65 + 64] = Wv[:, g * DO + h * 64 : g * DO + (h + 1) * 64]
            cache[g] = dict(
                wq=np.ascontiguousarray(Wq[:, sl]).astype(np.float16),
                wk=np.ascontiguousarray(Wk[:, sl]).astype(np.float16),
                wvp=wvp.astype(np.float16),
                wo=np.ascontiguousarray(Wo[sl, :]).astype(ml_dtypes.bfloat16),
                bq=np.ascontiguousarray(bq[sl].reshape(NJ, 128).T),
                bk=np.ascontiguousarray(bk[sl].reshape(NJ, 128).T),
                idn=idn,
            )
        m = dict(cache[g])
        m["xt"] = np.ascontiguousarray(x[b].T).astype(np.float16)
        import hashlib
        _nonce = 1 + int(hashlib.sha256(open(__file__, "rb").read()).hexdigest(), 16) % 509
        m["nonce"] = np.zeros((1, _nonce), np.float32)
        in_maps.append(m)
    return in_maps


_NC_CACHE = {}


def kernel(x, Wq, bq, Wk, bk, Wv, bv, Wo, bo):
    global LAST_EXEC_NS
    x = np.asarray(x, dtype=np.float32)
    Wq, bq = np.asarray(Wq, np.float32), np.asarray(bq, np.float32)
    Wk, bk = np.asarray(Wk, np.float32), np.asarray(bk, np.float32)
    Wv, bv = np.asarray(Wv, np.float32), np.asarray(bv, np.float32)
    Wo, bo = np.asarray(Wo, np.float32), np.asarray(bo, np.float32)
    B, NT, DM = x.shape

    from concourse.bass_utils import run_bass_kernel_spmd

    key = (NT, DM)
    if key not in _NC_CACHE:
        _NC_CACHE[key] = build_kernel(NT=NT, DM=DM)
    nc = _NC_CACHE[key]

    in_maps = shard_inputs(x, Wq, bq, Wk, bk, Wv, bv, Wo, bo)
    trace = bool(int(os.environ.get("BSATTN_TRACE", "0")))
    res = run_bass_kernel_spmd(nc, in_maps, list(range(N_CORES)), trace=trace)
    LAST_EXEC_NS = res.exec_time_ns
    globals()["LAST_RESULT"] = res

    out = np.empty((B, NT, DM), np.float32)
    # the V bias is folded out of the kernel: P@(V+bv)/denom = P@V/denom + bv
    # (softmax rows sum to 1), so its effect on y is the constant row bv@Wo
    yconst = bo + bv @ Wo
    for b in range(B):
        out[b] = res.results[2 * b]["y"] + res.results[2 * b + 1]["y"] + yconst
    return out


# revision 42
# speedup vs baseline: 1.0528x; 1.0528x over previous
"""TRN2 Bass kernel for block-sparse attention (nn_BlockSparseAttention).

kernel(**inputs) takes the FULL unsharded inputs (x [4,4096,1024], Wq/Wk/Wv/Wo
[1024,1024], bq/bk/bv/bo [1024]) and returns the full output [4,4096,1024].

Sharding: 8 cores = 4 batches x 2 head-halves (8 heads each). Each core
computes QKV projections, block-sparse attention, and a partial
out-projection [4096,1024]; the host sums the two half-partials plus bo.

v3 design (fused single pipeline):
  - projections and attention are interleaved over 8 token chunks of 512:
    the 512-col projection matmuls keep the PE HAM clock-gate at 8/8
    (2.4 GHz) and hide the Act exp / DVE division latency of the
    attention stages, which previously ran in a separate phase at K=4/8
  - S^T matmuls (64-row contraction) for the two head-halves are emitted
    interleaved so they run CONCURRENTLY in disjoint PE row-groups
    (tile_position auto-derived from base_partition) -- ~2x on S
  - no mask matmuls: every S matmul is its own accumulation group and the
    invalid 64x64 window corners are zeroed on the idle GpSimd engine
    after the exp
  - block-0 global attention is split into small (j, 4-piece) tasks spread
    across the whole pipeline; a few out-projections are deferred to keep
    the PE dense through the serial tail (g0 / edge group)
  - HAM pre-warm: dummy matmuls on the identity tile during the initial
    weight/x DMA wait
"""
import os

import numpy as np

import concourse.bass as bass
import concourse.tile as tile
from concourse import mybir

F32 = mybir.dt.float32
F16 = mybir.dt.float16
BF16 = mybir.dt.bfloat16
AF = mybir.ActivationFunctionType
SCALE = 1.0 / 8.0  # 1/sqrt(Dh=64)

N_CORES = 8
LAST_EXEC_NS = None


def _split_sync_waits(nc, cap=1):
    """This walrus build rejects >cap sync waits on one instruction; move
    excess waits onto same-engine no-ops placed just before (waits only
    become stricter in order, so this is semantics-preserving)."""
    for fn in nc.m.functions:
        for bb in fn.blocks:
            out = []
            for inst in bb.instructions:
                si = inst.sync_info
                waits = list(si.on_wait) if si and si.on_wait else []
                if len(waits) > cap:
                    extra, keep = waits[:-cap], waits[-cap:]
                    for i in range(0, len(extra), cap):
                        nop = mybir.InstNoOp(
                            name=nc.get_next_instruction_name(),
                            engine=inst.engine,
                            ins=[],
                            outs=[],
                            sync_info=mybir.SyncInfo(
                                on_wait=extra[i : i + cap], on_update=[]
                            ),
                        )
                        nc.register_instruction(nop)
                        out.append(nop)
                    si.on_wait = keep
                out.append(inst)
            bb.instructions[:] = out


def build_kernel(NT=4096, DM=1024, HL=8, DMO=1024):
    """One-core program; SPMD across 8 cores with different input slices."""
    DO = HL * 64          # local head dims (512)
    DOV = HL * 65         # v with interleaved ones columns (520)
    KC = DM // 128        # d_model chunks (8)
    NJ = DO // 128        # head pairs (4)
    STOK = 512
    NS = NT // STOK       # 8
    NG = NT // 128        # 32 token chunks / query groups
    NCH = NG + 1          # 33 shifted v chunks

    nc = bass.Bass()
    # cache-buster: some compile caches in this stack key on the HLO
    # interface only (not the embedded BIR), so a stale executable from a
    # previous kernel version can be wrongly reused. A source-hash-sized
    # dummy input makes every kernel edit change the HLO signature.
    import hashlib
    _nonce = 1 + int(hashlib.sha256(open(__file__, "rb").read()).hexdigest(), 16) % 509
    nonce_d = nc.dram_tensor("nonce", [1, _nonce], F32, kind="ExternalInput")
    xt_d = nc.dram_tensor("xt", [DM, NT], F16, kind="ExternalInput")
    wq_d = nc.dram_tensor("wq", [DM, DO], F16, kind="ExternalInput")
    wk_d = nc.dram_tensor("wk", [DM, DO], F16, kind="ExternalInput")
    wv_d = nc.dram_tensor("wvp", [DM, DOV], F16, kind="ExternalInput")
    wo_d = nc.dram_tensor("wo", [DO, DMO], BF16, kind="ExternalInput")
    bq_d = nc.dram_tensor("bq", [128, NJ], F32, kind="ExternalInput")
    bk_d = nc.dram_tensor("bk", [128, NJ], F32, kind="ExternalInput")
    idn_d = nc.dram_tensor("idn", [128, 128], BF16, kind="ExternalInput")
    y_d = nc.dram_tensor("y", [NT, DMO], F32, kind="ExternalOutput")

    with tile.TileContext(nc) as tc, nc.allow_low_precision(
        reason="attention operands intentionally bf16/fp16; matmul accum f32"
    ):
        from contextlib import ExitStack

        with ExitStack() as ctx:
            pers = ctx.enter_context(tc.tile_pool(name="pers", bufs=1))
            xp = ctx.enter_context(tc.tile_pool(name="xp", bufs=2))
            etp = ctx.enter_context(tc.tile_pool(name="etp", bufs=24))
            valp = ctx.enter_context(tc.tile_pool(name="valp", bufs=4))
            astp = ctx.enter_context(tc.tile_pool(name="astp", bufs=10))
            atp = ctx.enter_context(tc.tile_pool(name="atp", bufs=14))
            dip = ctx.enter_context(tc.tile_pool(name="dip", bufs=8))
            ysp = ctx.enter_context(tc.tile_pool(name="ysp", bufs=4))
            psp = ctx.enter_context(
                tc.tile_pool(name="psp", bufs=2, space="PSUM")
            )

            kts = [pers.tile([128, NT], BF16, tag=f"kt{j}", name=f"kt{j}")
                   for j in range(NJ)]
            qts = [pers.tile([128, NT], BF16, tag=f"qt{j}", name=f"qt{j}")
                   for j in range(NJ)]
            # shifted v: chunk m = tokens [128m-64, 128m+64); per-j cols 130
            vs = pers.tile([128, NCH * DOV], BF16, tag="vs")
            idn = pers.tile([128, 128], BF16, tag="idn")
            # block-0 PV accumulators (SBUF, accumulated task-wise)
            oq0 = [pers.tile([64, 130], F32, tag=f"oq{j}", name=f"oq{j}")
                   for j in range(NJ)]
            wqs = pers.tile([128, KC * DO], F16, tag="wqs")
            wks = pers.tile([128, KC * DO], F16, tag="wks")
            wvs = pers.tile([128, KC * DOV], F16, tag="wvs")
            bqs = pers.tile([128, NJ], F32, tag="bqs")
            bks = pers.tile([128, NJ], F32, tag="bks")
            wos = [pers.tile([128, DMO], BF16, tag=f"wo{j}", name=f"wo{j}")
                   for j in range(NJ)]

            # ---------------- prologue DMAs ----------------
            nc.sync.dma_start(idn[:], idn_d[:])
            xts_tiles = {}
            xts_tiles[0] = xp.tile([128, KC * STOK], F16, tag="xts",
                                   name="xts0")
            for c in range(KC):
                nc.sync.dma_start(
                    xts_tiles[0][:, c * STOK : (c + 1) * STOK],
                    xt_d[c * 128 : (c + 1) * 128, 0:STOK],
                )
            for c in range(KC):
                r = slice(c * 128, (c + 1) * 128)
                nc.sync.dma_start(wqs[:, c * DO : (c + 1) * DO], wq_d[r, :])
            nc.sync.dma_start(bqs[:], bq_d[:])
            for c in range(KC):
                r = slice(c * 128, (c + 1) * 128)
                nc.sync.dma_start(wks[:, c * DO : (c + 1) * DO], wk_d[r, :])
            nc.sync.dma_start(bks[:], bk_d[:])
            for c in range(KC):
                r = slice(c * 128, (c + 1) * 128)
                nc.sync.dma_start(wvs[:, c * DOV : (c + 1) * DOV], wv_d[r, :])
            for j in range(NJ):
                nc.sync.dma_start(wos[j][:], wo_d[j * 128 : (j + 1) * 128, :])
            # zero the phantom halves of the shifted-v layout so padded
            # full-row PV chains multiply by 0 instead of garbage
            nc.vector.memset(vs[0:64, 0:DOV], 0.0)
            nc.vector.memset(vs[64:128, NG * DOV : (NG + 1) * DOV], 0.0)

            # HAM pre-warm: ~3.4us of dummy matmuls during the DMA wait so
            # the first real chains run at K=8/8. Uses the first psY slot;
            # nothing reads it.
            NPW = int(os.environ.get("BSV3_PREWARM", "40"))
            if NPW:
                pw = psp.tile([128, 512], F32, tag="psYT", name="prewarm",
                              bufs=2)
                for i in range(NPW):
                    nc.tensor.matmul(pw[:, 0:128], idn[:], idn[:],
                                     start=(i == 0), stop=(i == NPW - 1))

            # ---------------- helpers ----------------
            def vcol(m, j, hh):
                return m * DOV + j * 130 + hh * 65

            ets_store = {}  # pair g1 -> {(j, hh): (tile, qoff)}
            as_tiles = {}   # g -> [ast per j]
            at_tiles = {}   # g -> at tile

            def emit_S2j(g1, j):
                """S^T for group pair (g1, g1+1), head pair j, both halves
                interleaved so the two 64-row chains run concurrently in
                disjoint PE row-groups. Tile layout per (j, hh):
                [ca x q1 | cm x q1 | cm x q2 | cb x q2], 128 cols each.
                Every matmul is its own accumulation group (no column
                overlap); the 4 invalid 64x64 corners are zeroed by GpSimd
                after the exp."""
                g2 = g1 + 1
                ka = 128 * g1 - 64
                km = 128 * g1 + 64
                kb = 128 * g2 + 64
                ps = {}
                for hh in (0, 1):
                    ps[hh] = psp.tile([128, 512], F32, tag="psS",
                                      name=f"psS_{g1}_{j}_{hh}")
                if os.environ.get("BSV3_SEQS2"):
                    order = [(seg, hh) for hh in (0, 1) for seg in range(3)]
                else:
                    order = [(seg, hh) for seg in range(3) for hh in (0, 1)]
                segs = ((ka, 0, 128), (km, 0, 256), (kb, 128, 128))
                for seg, hh in order:
                    ko, qo, qn = segs[seg]
                    co = (0, 128, 384)[seg]
                    hr = slice(hh * 64, hh * 64 + 64)
                    nc.tensor.matmul(
                        ps[hh][:, co : co + qn],
                        kts[j][hr, ko : ko + 128],
                        qts[j][hr, 128 * g1 + qo : 128 * g1 + qo + qn],
                        start=True, stop=True,
                    )
                st = ets_store.setdefault(g1, {})
                MS = os.environ.get("BSV3_MEMSET", "gpsimd")
                for hh in (0, 1):
                    et = etp.tile([128, 512], BF16, tag="et",
                                  name=f"et_{g1}_{j}_{hh}")
                    nc.scalar.activation(et[:], ps[hh][:], AF.Exp,
                                         scale=SCALE)
                    eng = {"gpsimd": nc.gpsimd, "vector": nc.vector,
                           "none": None}[MS]
                    if eng is not None:
                        eng.memset(et[0:64, 64:128], 0.0)
                        eng.memset(et[64:128, 128:192], 0.0)
                        eng.memset(et[0:64, 320:384], 0.0)
                        eng.memset(et[64:128, 384:448], 0.0)
                    st[(j, hh)] = (et, 0)
                    st[(j, hh, "g2")] = (et, 256)

            def emit_S31j(j):
                """Edge group 31 (no right neighbor), head pair j. Layout
                per hh half-tile: [ca x q | cb x q], cb only 64 keys."""
                g = NG - 1
                ka = 128 * g - 64
                kb = 128 * g + 64
                ps = {}
                for hh in (0, 1):
                    ps[hh] = psp.tile([128, 512], F32, tag="psS",
                                      name=f"psS31_{j}_{hh}")
                # adjacent matmuls in disjoint row-groups run concurrently
                # and must drain to different PSUM banks
                for hh in (0, 1):
                    hr = slice(hh * 64, hh * 64 + 64)
                    nc.tensor.matmul(
                        ps[hh][:, 0:128],
                        kts[j][hr, ka : ka + 128],
                        qts[j][hr, 128 * g : 128 * g + 128],
                        start=True, stop=True,
                    )
                for hh in (0, 1):
                    hr = slice(hh * 64, hh * 64 + 64)
                    nc.tensor.matmul(
                        ps[hh][0:64, 128:256],
                        kts[j][hr, kb : kb + 64],
                        qts[j][hr, 128 * g : 128 * g + 128],
                        start=True, stop=True,
                    )
                st = ets_store.setdefault("e31", {})
                et = etp.tile([128, 512], BF16, tag="et", name=f"et31_{j}")
                for hh in (0, 1):
                    o = hh * 256
                    nc.scalar.activation(et[:, o : o + 128],
                                         ps[hh][:, 0:128],
                                         AF.Exp, scale=SCALE)
                    nc.scalar.activation(et[0:64, o + 128 : o + 256],
                                         ps[hh][0:64, 128:256],
                                         AF.Exp, scale=SCALE)
                    nc.gpsimd.memset(et[0:64, o + 64 : o + 128], 0.0)
                    nc.gpsimd.memset(et[64:128, o + 128 : o + 256], 0.0)
                    st[(j, hh)] = (et, hh * 256)
                return st

            def emit_PV(g, ets):
                """merged PV, batched reciprocal, division on DVE."""
                as_tiles[g] = []
                for j in range(NJ):
                    jj = j % 2
                    if jj == 0:
                        pv = psp.tile([128, 512], F32, tag="psPV",
                                      name=f"psPV_{g}_{j}")
                    c0 = jj * 130
                    for hh in (0, 1):
                        et, off = ets[(j, hh)]
                        co = c0 + hh * 65
                        nc.tensor.matmul(
                            pv[:, co : co + 65],
                            et[:, off : off + 128],
                            vs[:, vcol(g, j, hh) : vcol(g, j, hh) + 65],
                            start=True, stop=False,
                        )
                        nc.tensor.matmul(
                            pv[:, co : co + 65],
                            et[:, off + 128 : off + 256],
                            vs[:, vcol(g + 1, j, hh) : vcol(g + 1, j, hh) + 65],
                            start=False, stop=True,
                        )
                    if jj == 1:
                        dinv = dip.tile([128, 4], F32, tag="dinv",
                                        name=f"dinv_{g}_{j}")
                        nc.vector.reciprocal(
                            dinv[:],
                            pv[:, 0:260].rearrange(
                                "p (h c) -> p h c", c=65
                            )[:, :, 64:65],
                        )
                        for j2 in (j - 1, j):
                            ast = astp.tile([128, 128], BF16, tag="ast",
                                            name=f"ast_{g}_{j2}")
                            cb = (j2 % 2) * 130
                            for hh in (0, 1):
                                nc.vector.tensor_scalar_mul(
                                    ast[:, hh * 64 : hh * 64 + 64],
                                    pv[:, cb + hh * 65 : cb + hh * 65 + 64],
                                    dinv[:, (j2 % 2) * 2 + hh : (j2 % 2) * 2 + hh + 1],
                                )
                            as_tiles[g].append(ast)

            def emit_T(g):
                """PE-transpose astage -> one A^T tile [128, 4*128]."""
                pt = psp.tile([128, 1024], BF16, tag="psYT",
                              name=f"psT_{g}", bufs=2)
                for j in range(NJ):
                    nc.tensor.transpose(
                        pt[:, j * 128 : (j + 1) * 128], as_tiles[g][j], idn
                    )
                att = atp.tile([128, 512], BF16, tag="at", name=f"at_{g}")
                nc.vector.tensor_copy(att[:], pt[:, 0:512])
                at_tiles[g] = att
                del as_tiles[g]

            def emit_outproj_half(g, n):
                py = psp.tile([128, 512], F32, tag="psYT",
                              name=f"psY_{g}_{n}", bufs=2)
                for j in range(NJ):
                    nc.tensor.matmul(
                        py[:],
                        at_tiles[g][:, j * 128 : (j + 1) * 128],
                        wos[j][:, n * 512 : n * 512 + 512],
                        start=(j == 0), stop=(j == NJ - 1),
                    )
                ysb = ysp.tile([128, 512], F32, tag="ysb", name=f"ysb_{g}_{n}")
                if (g + n) % 2 == 0:
                    nc.scalar.copy(ysb[:], py[:])
                else:
                    nc.vector.tensor_copy(ysb[:], py[:])
                nc.sync.dma_start(
                    y_d[g * 128 : (g + 1) * 128, n * 512 : n * 512 + 512],
                    ysb[:],
                )
                if n == 1:
                    del at_tiles[g]

            # ----- block-0 global attention tasks -----
            def q0_edge0(j):
                """piece 0 (keys 0:64 live on partitions 64:128 of chunk 0);
                initializes oq0[j] via tensor_copy."""
                ps = {}
                for hh in (0, 1):
                    ps[hh] = psp.tile([128, 512], F32, tag="psS",
                                      name=f"q0e0_{j}_{hh}")
                    hr = slice(hh * 64, hh * 64 + 64)
                    nc.tensor.matmul(
                        ps[hh][64:128, 0:64],
                        kts[j][hr, 0:64], qts[j][hr, 0:64],
                        start=True, stop=True,
                    )
                eq = etp.tile([128, 512], BF16, tag="et", name=f"eq0_{j}")
                for hh in (0, 1):
                    nc.scalar.activation(eq[64:128, hh * 64 : hh * 64 + 64],
                                         ps[hh][64:128, 0:64],
                                         AF.Exp, scale=SCALE)
                pv = psp.tile([128, 512], F32, tag="psPV", name=f"pvq0_{j}")
                for hh in (0, 1):
                    nc.tensor.matmul(
                        pv[0:64, hh * 65 : hh * 65 + 65],
                        eq[64:128, hh * 64 : hh * 64 + 64],
                        vs[64:128, vcol(0, j, hh) : vcol(0, j, hh) + 65],
                        start=True, stop=True,
                    )
                nc.vector.tensor_copy(oq0[j][:], pv[0:64, 0:130])

            def q0_edge32(j):
                """piece 32 (keys 4032:4096 on partitions 0:64 of chunk 32)."""
                ps = {}
                for hh in (0, 1):
                    ps[hh] = psp.tile([128, 512], F32, tag="psS",
                                      name=f"q0e32_{j}_{hh}")
                    hr = slice(hh * 64, hh * 64 + 64)
                    nc.tensor.matmul(
                        ps[hh][0:64, 0:64],
                        kts[j][hr, NT - 64 : NT], qts[j][hr, 0:64],
                        start=True, stop=True,
                    )
                eq = etp.tile([128, 512], BF16, tag="et", name=f"eq32_{j}")
                for hh in (0, 1):
                    nc.scalar.activation(eq[0:64, hh * 64 : hh * 64 + 64],
                                         ps[hh][0:64, 0:64],
                                         AF.Exp, scale=SCALE)
                pv = psp.tile([128, 512], F32, tag="psPV", name=f"pvq32_{j}")
                for hh in (0, 1):
                    nc.tensor.matmul(
                        pv[0:64, hh * 65 : hh * 65 + 65],
                        eq[0:64, hh * 64 : hh * 64 + 64],
                        vs[0:64, vcol(NG, j, hh) : vcol(NG, j, hh) + 65],
                        start=True, stop=True,
                    )
                nc.vector.tensor_add(oq0[j][:], oq0[j][:], pv[0:64, 0:130])

            def q0_range(j, r):
                """block-0 vs pieces 4r+1..4r+4 (clipped to 31) for head
                pair j. S matmuls packed across hh row-groups; one exp; PV
                chains per hh accumulated into oq0[j]."""
                pieces = [m for m in range(4 * r + 1, 4 * r + 5) if m <= NG - 1]
                ps = {}
                for hh in (0, 1):
                    ps[hh] = psp.tile([128, 512], F32, tag="psS",
                                      name=f"q0r_{j}_{r}_{hh}")
                for c, m in enumerate(pieces):
                    for hh in (0, 1):
                        hr = slice(hh * 64, hh * 64 + 64)
                        nc.tensor.matmul(
                            ps[hh][:, c * 64 : c * 64 + 64],
                            kts[j][hr, 128 * m - 64 : 128 * m + 64],
                            qts[j][hr, 0:64],
                            start=True, stop=True,
                        )
                eq = etp.tile([128, 512], BF16, tag="et", name=f"eqr_{j}_{r}")
                nw = len(pieces) * 64
                for hh in (0, 1):
                    nc.scalar.activation(
                        eq[:, hh * 256 : hh * 256 + nw],
                        ps[hh][:, 0:nw],
                        AF.Exp, scale=SCALE,
                    )
                pv = psp.tile([128, 512], F32, tag="psPV", name=f"pvr_{j}_{r}")
                for hh in (0, 1):
                    for c, m in enumerate(pieces):
                        nc.tensor.matmul(
                            pv[0:64, hh * 65 : hh * 65 + 65],
                            eq[:, hh * 256 + c * 64 : hh * 256 + c * 64 + 64],
                            vs[:, vcol(m, j, hh) : vcol(m, j, hh) + 65],
                            start=(c == 0), stop=(c == len(pieces) - 1),
                        )
                nc.vector.tensor_add(oq0[j][:], oq0[j][:], pv[0:64, 0:130])

            def emit_g0():
                """group 0: block 0 (rows 0:64, from the accumulated global
                pass) + block 1 (rows 64:128, local window {0,1,2})."""
                as_tiles[0] = []
                ega, egb = {}, {}
                for hh in (0, 1):
                    hr = slice(hh * 64, hh * 64 + 64)
                    psA = psp.tile([128, 512], F32, tag="psS",
                                   name=f"g0a_{hh}")
                    for j in range(NJ):
                        nc.tensor.matmul(
                            psA[64:128, j * 64 : j * 64 + 64],
                            kts[j][hr, 0:64], qts[j][hr, 64:128],
                            start=True, stop=True,
                        )
                        nc.tensor.matmul(
                            psA[:, 256 + j * 64 : 256 + j * 64 + 64],
                            kts[j][hr, 64:192], qts[j][hr, 64:128],
                            start=True, stop=True,
                        )
                    ea = etp.tile([128, 512], BF16, tag="et", name=f"ga_{hh}")
                    nc.scalar.activation(
                        ea[64:128, 0:256], psA[64:128, 0:256],
                        AF.Exp, scale=SCALE,
                    )
                    nc.scalar.activation(ea[:, 256:512], psA[:, 256:512],
                                         AF.Exp, scale=SCALE)
                    nc.gpsimd.memset(ea[0:64, 0:256], 0.0)
                    ega[hh], egb[hh] = ea, ea
                for j in range(NJ):
                    jj = j % 2
                    if jj == 0:
                        pv0 = psp.tile([128, 512], F32, tag="psPV",
                                       name=f"pv0_{j}")
                    c0 = jj * 130
                    for hh in (0, 1):
                        nc.tensor.matmul(
                            pv0[64:128, c0 + hh * 65 : c0 + hh * 65 + 65],
                            ega[hh][:, j * 64 : j * 64 + 64],
                            vs[:, vcol(0, j, hh) : vcol(0, j, hh) + 65],
                            start=True, stop=False,
                        )
                        nc.tensor.matmul(
                            pv0[64:128, c0 + hh * 65 : c0 + hh * 65 + 65],
                            egb[hh][:, 256 + j * 64 : 256 + j * 64 + 64],
                            vs[:, vcol(1, j, hh) : vcol(1, j, hh) + 65],
                            start=False, stop=True,
                        )
                    dinv = dip.tile([128, 4], F32, tag="dinv",
                                    name=f"dinv0_{j}")
                    for hh in (0, 1):
                        nc.vector.reciprocal(
                            dinv[64:128, hh : hh + 1],
                            pv0[64:128, c0 + hh * 65 + 64 : c0 + hh * 65 + 65],
                        )
                        nc.vector.reciprocal(
                            dinv[0:64, hh : hh + 1],
                            oq0[j][:, hh * 65 + 64 : hh * 65 + 65],
                        )
                    ast = astp.tile([128, 128], BF16, tag="ast",
                                    name=f"ast0_{j}")
                    for hh in (0, 1):
                        nc.vector.tensor_scalar_mul(
                            ast[64:128, hh * 64 : hh * 64 + 64],
                            pv0[64:128, c0 + hh * 65 : c0 + hh * 65 + 64],
                            dinv[64:128, hh : hh + 1],
                        )
                        nc.vector.tensor_scalar_mul(
                            ast[0:64, hh * 64 : hh * 64 + 64],
                            oq0[j][:, hh * 65 : hh * 65 + 64],
                            dinv[0:64, hh : hh + 1],
                        )
                    as_tiles[0].append(ast)

            # ---------------- task scheduler ----------------
            # slot L = 16*s + i; tasks emit after the slot's proj chain when
            # ready_L <= L. Insertion order respects intra-group deps.
            tasks = []
            LEVEL = int(os.environ.get("BSV3_LEVEL", "99"))

            def add(ready, cost, fn, lvl=0):
                if lvl > LEVEL:
                    return
                tasks.append([ready, cost, fn, False])

            def pump(L, budget):
                spent = 0.0
                for t in tasks:
                    if t[3] or t[0] > L:
                        continue
                    t[2]()
                    t[3] = True
                    spent += t[1]
                    if spent >= budget:
                        break

            def vslot(T):
                """slot index after which vs chunk T's scatter DMA is
                emitted (+1 slot of lag for the DMA to land)."""
                s, t = T // 4, T % 4
                return 16 * s + 10 + 2 * t

            DEFER = {21, 22, 23, 24}


            def pv1_ets(g1):
                return {k: v for k, v in ets_store[g1].items()
                        if len(k) == 2}

            def pv2_ets(g1):
                return {(j, hh): ets_store[g1][(j, hh, "g2")]
                        for j in range(NJ) for hh in (0, 1)}

            # pair tasks
            for g1 in range(1, NG - 1, 2):
                g2 = g1 + 1
                sready = 16 * ((g2 + 1) // 4) + 8
                for j in range(NJ):
                    add(sready + j // 2, 0.45,
                        (lambda g1=g1, j=j: emit_S2j(g1, j)), lvl=1)
                pvr1 = max(sready + 3, vslot(g1 + 1) + 1)
                pvr2 = max(pvr1 + 1, vslot(g2 + 1) + 1)
                add(pvr1, 0.6,
                    (lambda g1=g1: emit_PV(g1, pv1_ets(g1))), lvl=2)
                add(pvr2, 0.6,
                    (lambda g1=g1, g2=g2: emit_PV(g2, pv2_ets(g1))), lvl=2)
                add(pvr2 + 1, 0.3, (lambda g1=g1: emit_T(g1)), lvl=3)
                add(pvr2 + 1, 0.3, (lambda g2=g2: emit_T(g2)), lvl=3)
                for gg, base in ((g1, pvr2 + 3), (g2, pvr2 + 4)):
                    if gg in DEFER:
                        base = 129 + 2 * (gg - 21)
                    add(base, 0.5,
                        (lambda gg=gg: emit_outproj_half(gg, 0)), lvl=4)
                    add(base + 1, 0.5,
                        (lambda gg=gg: emit_outproj_half(gg, 1)), lvl=4)

            # edge group 31
            S31LVL = 9 if os.environ.get("BSV3_NOS31") else 1
            for j in range(NJ):
                add(16 * 7 + 8 + j // 2, 0.35, (lambda j=j: emit_S31j(j)),
                    lvl=S31LVL)
            add(vslot(NG - 1) + 2, 0.6, (lambda: emit_PV(
                NG - 1, {k: v for k, v in ets_store["e31"].items()})), lvl=2)
            add(vslot(NG - 1) + 3, 0.3, (lambda: emit_T(NG - 1)), lvl=3)
            add(vslot(NG - 1) + 4, 0.5,
                (lambda: emit_outproj_half(NG - 1, 0)), lvl=4)
            add(vslot(NG - 1) + 5, 0.5,
                (lambda: emit_outproj_half(NG - 1, 1)), lvl=4)

            # block-0 tasks
            for j in range(NJ):
                add(11 + j // 2, 0.2, (lambda j=j: q0_edge0(j)), lvl=5)
            for r in range(8):
                for j in range(NJ):
                    rd = max(16 * min(r + 1, NS - 1) + 8,
                             vslot(min(4 * r + 4, NG - 1)) + 1)
                    add(rd + j, 0.45, (lambda j=j, r=r: q0_range(j, r)),
                        lvl=5)
            for j in range(NJ):
                add(vslot(NG - 1) + 2 + j, 0.2, (lambda j=j: q0_edge32(j)),
                    lvl=5)
            add(137, 2.0, emit_g0, lvl=6)
            add(138, 0.3, (lambda: emit_T(0)), lvl=6)
            add(139, 0.5, (lambda: emit_outproj_half(0, 0)), lvl=6)
            add(140, 0.5, (lambda: emit_outproj_half(0, 1)), lvl=6)

            # ---------------- the fused pipeline ----------------
            for s in range(NS):
                ts = slice(s * STOK, (s + 1) * STOK)
                L0 = 16 * s
                if s + 1 < NS:
                    xts_tiles[s + 1] = xp.tile([128, KC * STOK], F16,
                                               tag="xts", name=f"xts{s + 1}")
                    for c in range(KC):
                        nc.sync.dma_start(
                            xts_tiles[s + 1][:, c * STOK : (c + 1) * STOK],
                            xt_d[c * 128 : (c + 1) * 128,
                                 (s + 1) * STOK : (s + 2) * STOK],
                        )
                xts = xts_tiles.pop(s)
                for qi, (wsb, bsb, dsts) in enumerate(
                    ((wqs, bqs, qts), (wks, bks, kts))
                ):
                    for j in range(NJ):
                        ps = psp.tile([128, 512], F32, tag="pp",
                                      name=f"pp_{s}_{qi}_{j}")
                        for c in range(KC):
                            nc.tensor.matmul(
                                ps[:],
                                wsb[:, c * DO + j * 128 : c * DO + (j + 1) * 128],
                                xts[:, c * STOK : (c + 1) * STOK],
                                start=(c == 0), stop=(c == KC - 1),
                            )
                        nc.scalar.activation(
                            dsts[j][:, ts], ps[:], AF.Identity,
                            bias=bsb[:, j : j + 1],
                        )
                        pump(L0 + 4 * qi + j, 1.1)
                for t in range(STOK // 128):
                    T = 4 * s + t
                    val = valp.tile([128, DOV], BF16, tag="val",
                                    name=f"val_{T}")
                    for oi, (o, wd) in enumerate(((0, 260), (260, 260))):
                        psv = psp.tile([128, 512], F32, tag="pp",
                                       name=f"ppv_{T}_{oi}")
                        for c in range(KC):
                            nc.tensor.matmul(
                                psv[:, 0:wd],
                                xts[:, c * STOK + t * 128 : c * STOK + (t + 1) * 128],
                                wvs[:, c * DOV + o : c * DOV + o + wd],
                                start=(c == 0), stop=(c == KC - 1),
                            )
                        nc.vector.tensor_copy(val[:, o : o + wd],
                                              psv[:, 0:wd])
                        pump(L0 + 8 + 2 * t + oi, 0.9)
                    nc.gpsimd.memset(
                        val.rearrange("p (h c) -> p h c", c=65)[:, :, 64:65],
                        1.0,
                    )
                    # scatter into shifted-chunk layout
                    nc.sync.dma_start(
                        vs[64:128, T * DOV : (T + 1) * DOV], val[0:64, :]
                    )
                    nc.sync.dma_start(
                        vs[0:64, (T + 1) * DOV : (T + 2) * DOV], val[64:128, :]
                    )
            # ---------------- tail ----------------
            L = 16 * NS
            while any(not t[3] for t in tasks):
                pump(L, 2.2)
                L += 1
                assert L < 400, "scheduler deadlock"

    return _finish(nc)


def _finish(nc):
    _split_sync_waits(nc)
    return nc


# ---------------------------------------------------------------- host glue
def shard_inputs(x, Wq, bq, Wk, bk, Wv, bv, Wo, bo):
    """Full inputs -> per-core in_maps. Core c: batch c//2, head-half c%2."""
    import ml_dtypes

    DM = Wq.shape[0]
    DO = Wq.shape[1] // 2
    HL = DO // 64
    DOV = HL * 65
    NJ = DO // 128
    in_maps = []
    cache = {}
    idn = np.eye(128, dtype=ml_dtypes.bfloat16)
    for core in range(N_CORES):
        b, g = core // 2, core % 2
        if g not in cache:
            sl = slice(g * DO, (g + 1) * DO)
            wvp = np.zeros((DM, DOV), np.float32)
            for h in range(HL):
                wvp[:, h * 65 : h * 65 + 64] = Wv[:, g * DO + h * 64 : g * DO + (h + 1) * 64]
            cache[g] = dict(
                wq=np.ascontiguousarray(Wq[:, sl]).astype(np.float16),
                wk=np.ascontiguousarray(Wk[:, sl]).astype(np.float16),
                wvp=wvp.astype(np.float16),
                wo=np.ascontiguousarray(Wo[sl, :]).astype(ml_dtypes.bfloat16),
                bq=np.ascontiguousarray(bq[sl].reshape(NJ, 128).T),
                bk=np.ascontiguousarray(bk[sl].reshape(NJ, 128).T),
                idn=idn,
            )
        m = dict(cache[g])
        m["xt"] = np.ascontiguousarray(x[b].T).astype(np.float16)
        import hashlib
        _nonce = 1 + int(hashlib.sha256(open(__file__, "rb").read()).hexdigest(), 16) % 509
        m["nonce"] = np.zeros((1, _nonce), np.float32)
        in_maps.append(m)
    return in_maps


_NC_CACHE = {}


def kernel(x, Wq, bq, Wk, bk, Wv, bv, Wo, bo):
    global LAST_EXEC_NS
    x = np.asarray(x, dtype=np.float32)
    Wq, bq = np.asarray(Wq, np.float32), np.asarray(bq, np.float32)
    Wk, bk = np.asarray(Wk, np.float32), np.asarray(bk, np.float32)
    Wv, bv = np.asarray(Wv, np.float32), np.asarray(bv, np.float32)
    Wo, bo = np.asarray(Wo, np.float32), np.asarray(bo, np.float32)
    B, NT, DM = x.shape

    from concourse.bass_utils import run_bass_kernel_spmd

    key = (NT, DM)
    if key not in _NC_CACHE:
        _NC_CACHE[key] = build_kernel(NT=NT, DM=DM)
    nc = _NC_CACHE[key]

    in_maps = shard_inputs(x, Wq, bq, Wk, bk, Wv, bv, Wo, bo)
    trace = bool(int(os.environ.get("BSATTN_TRACE", "0")))
    res = run_bass_kernel_spmd(nc, in_maps, list(range(N_CORES)), trace=trace)
    LAST_EXEC_NS = res.exec_time_ns
    globals()["LAST_RESULT"] = res

    out = np.empty((B, NT, DM), np.float32)
    # the V bias is folded out of the kernel: P@(V+bv)/denom = P@V/denom + bv
    # (softmax rows sum to 1), so its effect on y is the constant row bv@Wo
    yconst = bo + bv @ Wo
    for b in range(B):
        out[b] = res.results[2 * b]["y"] + res.results[2 * b + 1]["y"] + yconst
    return out
